# revision 1
# baseline (speedup 1.0000x reference)
"""Integrate-and-fire scan (T=8) on Trainium2, data-parallel over 8 NeuronCores.

Reference semantics per element, scanned over t:
    mem = mem + x[t]; spike = (mem - 1 > 0); mem = mem - spike

Sharding: batch dim (axis 1 of x / axis 0 of mem0) split 4-per-core across 8
cores; the scan is elementwise so no cross-core communication is needed.

Per core the shard is viewed as [T=8, P=128, F=4704] (4*3*224*224 = 602112 =
128*4704). The kernel streams column chunks: membrane chunk stays resident in
SBUF while the 8 timesteps are applied (TT add, TS is_gt, TT sub on VectorE),
spikes DMA out per timestep. DMA-bound: ~41 MB HBM traffic per core.
"""

import sys

if "/opt/trn_rl_repo" not in sys.path:
    sys.path.insert(0, "/opt/trn_rl_repo")

import numpy as np

import concourse.bass as bass  # noqa: F401  (registers engine classes)
import concourse.tile as tile
from concourse import bacc, mybir
from concourse.bass_utils import run_bass_kernel_spmd

T, B, C, H, W = 8, 32, 3, 224, 224
NCORES = 8
BPC = B // NCORES            # 4 batch elements per core
E = BPC * C * H * W          # 602112 elements per (core, timestep)
P = 128
F = E // P                   # 4704 free-dim columns
F32 = mybir.dt.float32

# Tunables
CHUNK_W = 1176               # columns per chunk (divides 4704)
N_CHUNKS = F // CHUNK_W
X_BUFS = 6                   # x-tile double buffering depth
S_BUFS = 6                   # spike-tile buffering depth

_compiled_nc = None


def _build():
    nc = bacc.Bacc("TRN2", target_bir_lowering=False, debug=False,
                   num_devices=NCORES)
    x = nc.dram_tensor("x", [T, P, F], F32, kind="ExternalInput").ap()
    m0 = nc.dram_tensor("mem0", [P, F], F32, kind="ExternalInput").ap()
    out = nc.dram_tensor("out", [T, P, F], F32, kind="ExternalOutput").ap()

    with tile.TileContext(nc) as tc:
        with tc.tile_pool(name="mem", bufs=N_CHUNKS) as mem_pool, \
             tc.tile_pool(name="xin", bufs=X_BUFS) as x_pool, \
             tc.tile_pool(name="spk", bufs=S_BUFS) as s_pool:
            for c in range(N_CHUNKS):
                sl = bass.ts(c, CHUNK_W)
                mt = mem_pool.tile([P, CHUNK_W], F32)
                nc.sync.dma_start(out=mt[:], in_=m0[:, sl])
                for t in range(T):
                    xt = x_pool.tile([P, CHUNK_W], F32)
                    nc.sync.dma_start(out=xt[:], in_=x[t, :, sl])
                    nc.vector.tensor_add(mt[:], mt[:], xt[:])
                    st = s_pool.tile([P, CHUNK_W], F32)
                    nc.vector.tensor_scalar(
                        out=st[:], in0=mt[:], scalar1=1.0, scalar2=None,
                        op0=mybir.AluOpType.is_gt)
                    nc.vector.tensor_sub(mt[:], mt[:], st[:])
                    nc.scalar.dma_start(out=out[t, :, sl], in_=st[:])
    nc.compile()
    return nc


def _get_nc():
    global _compiled_nc
    if _compiled_nc is None:
        _compiled_nc = _build()
    return _compiled_nc


def _run(x, mem0, trace=False):
    nc = _get_nc()
    in_maps = []
    for i in range(NCORES):
        bsl = slice(i * BPC, (i + 1) * BPC)
        xi = np.ascontiguousarray(x[:, bsl]).reshape(T, P, F)
        mi = np.ascontiguousarray(mem0[bsl]).reshape(P, F)
        in_maps.append({"x": xi, "mem0": mi})
    res = run_bass_kernel_spmd(nc, in_maps, list(range(NCORES)), trace=trace)
    shards = [res.results[i]["out"].reshape(T, BPC, C, H, W)
              for i in range(NCORES)]
    full = np.concatenate(shards, axis=1)
    return full, res


def kernel(x, mem0):
    x = np.asarray(x, dtype=np.float32)
    mem0 = np.asarray(mem0, dtype=np.float32)
    full, _ = _run(x, mem0, trace=False)
    return full
